# revision 1
# baseline (speedup 1.0000x reference)
"""AttentionWithMSR Trainium2 kernel — 8-core SPMD, data-parallel over (batch, H-half).

Self-contained: takes FULL inputs, shards internally, returns FULL output.

Math (reference):
    msr  = log1p(x) - (1/3) * sum_s log1p(blur_s(x)),  s in {15, 80, 250}
    a    = BN(conv1x1(g;  Wg)),  b = BN(conv1x1(msr; Wx))
    psi  = sigmoid(BN(conv1x1(relu(a + b); wpsi)))
    out  = x * psi

Kernel mapping:
  * blur_s(x) per (b, c) image M as two 256x256 matmuls: blur = G_s @ M @ G_s
    (G_s symmetric Toeplitz from the zero-padded normalized 1D Gaussian).
    Each core owns one batch sample b and one 128-row H-half:
      pass A (vertical, transposed): Vt = M^T @ GvT    [w(256, 2 chunks), h_own(128)]
      pass B (horizontal):           Blur = Vt^T @ G_s [h_own(128), w(256)]
  * BN folded into conv weights on host. msr folded into the conv:
      a+b = W1^T @ [g; log1p(x)] + W2^T @ [l12; l3] + bias0,  W2 = -[Wx; Wx]/3.
    l12/l3 round-trip through DRAM to convert per-channel [h, w] tiles into
    [channel, pixel] layout for the channel-contracting conv matmuls.
  * psi conv replicates its scalar output over 64 partitions inside the
    matmul (replicated-weight lhsT) so sigmoid/multiply run full-width.
"""

import sys

sys.path.insert(0, "/opt/trn_rl_repo")

import numpy as np
import ml_dtypes

SCALES = (15, 80, 250)
EPS = 1e-5
B, C, H, W = 4, 64, 256, 256
HALF = 128
FINT = 32
N_CORES = 8
BF16 = ml_dtypes.bfloat16

_CACHE = {}
_LAST_IN_MAPS = None


def _gauss_mat(scale: int) -> np.ndarray:
    """256x256 matrix of the zero-padded 'same' normalized 1D Gaussian blur."""
    k = int(4 * scale + 1)
    p = k // 2
    coords = np.arange(k, dtype=np.float32) - (k - 1) / 2.0
    g1 = np.exp(-(coords**2) / np.float32(2.0 * scale * scale))
    g1 = g1 / g1.sum()
    i = np.arange(W)
    D = i[None, :] - i[:, None]  # j - i
    M = np.where(np.abs(D) <= p, g1[np.clip(D + p, 0, k - 1)], np.float32(0.0))
    return M.astype(np.float32)


def _build_nc():
    import concourse.mybir as mybir
    import concourse.tile as tile
    from concourse import bacc

    bf = mybir.dt.bfloat16
    f32 = mybir.dt.float32
    AF = mybir.ActivationFunctionType

    nc = bacc.Bacc("TRN2", target_bir_lowering=False)

    # x[b] transposed to (h-chunk, h-in-chunk, channel, w) so SBUF loads get
    # 4KB-contiguous per-partition runs.
    xbt_e = nc.dram_tensor("xbt", [2, HALF, C, W], bf, kind="ExternalInput")
    xh_e = nc.dram_tensor("xh", [C, HALF * W], bf, kind="ExternalInput")
    gb_e = nc.dram_tensor("gb", [C, HALF * W], bf, kind="ExternalInput")
    lxh_e = nc.dram_tensor("lxh", [C, HALF * W], bf, kind="ExternalInput")
    gvt_e = nc.dram_tensor("gvt", [128, 2, 384], bf, kind="ExternalInput")
    gh_e = nc.dram_tensor("gh", [128, 6, W], bf, kind="ExternalInput")
    w1_e = nc.dram_tensor("w1", [128, FINT], bf, kind="ExternalInput")
    w2_e = nc.dram_tensor("w2", [128, FINT], bf, kind="ExternalInput")
    wpsi_e = nc.dram_tensor("wpsi", [128, 64], bf, kind="ExternalInput")
    bias0_e = nc.dram_tensor("bias0", [128, 1], f32, kind="ExternalInput")
    bpsi_e = nc.dram_tensor("bpsi", [128, 1], f32, kind="ExternalInput")
    out_e = nc.dram_tensor("out", [C, HALF * W], f32, kind="ExternalOutput")

    NPIX = HALF * W  # 32768 pixels per core

    with tile.TileContext(nc) as tc:
        with (
            tc.tile_pool(name="consts", bufs=1) as consts,
            tc.tile_pool(name="dram", bufs=1, space="DRAM") as dpool,
        ):
            gvt_sb = consts.tile([128, 2, 384], bf)
            nc.sync.dma_start(gvt_sb[:], gvt_e[:])
            gh_sb = consts.tile([128, 6, W], bf)
            nc.sync.dma_start(gh_sb[:], gh_e[:])
            w1_sb = consts.tile([128, FINT], bf)
            nc.sync.dma_start(w1_sb[:], w1_e[:])
            w2_sb = consts.tile([128, FINT], bf)
            nc.sync.dma_start(w2_sb[:], w2_e[:])
            wpsi_sb = consts.tile([128, 64], bf)
            nc.sync.dma_start(wpsi_sb[:], wpsi_e[:])
            bias0_sb = consts.tile([128, 1], f32)
            nc.sync.dma_start(bias0_sb[:], bias0_e[:])
            bpsi_sb = consts.tile([128, 1], f32)
            nc.sync.dma_start(bpsi_sb[:], bpsi_e[:])

            rhs1_all = consts.tile([128, NPIX], bf)  # [g; log1p(x)] conv rhs

            l12d = dpool.tile([C, NPIX], bf)
            l3d = dpool.tile([C, NPIX], bf)
            l12d_v = l12d[:].rearrange("c (h w) -> c h w", h=HALF)
            l3d_v = l3d[:].rearrange("c (h w) -> c h w", h=HALF)

            # ---- phase 1: blur + log1p, software-pipelined over channels
            with (
                tc.tile_pool(name="p1", bufs=3) as p1,
                tc.tile_pool(name="p1x", bufs=2) as p1x,
                tc.tile_pool(name="p1vt", bufs=2, space="PSUM") as p1vt,
                tc.tile_pool(name="p1bl", bufs=3, space="PSUM") as p1bl,
            ):
                GRP = 8  # channels per staged load
                xs_tiles = {}
                vt_tiles = {}

                def load_group(g):
                    xs_g = p1x.tile([128, 2, GRP, W], bf, tag="xs")
                    for hc in range(2):
                        nc.sync.dma_start(
                            xs_g[:, hc], xbt_e[hc, :, g * GRP : (g + 1) * GRP, :]
                        )
                    xs_tiles[g] = xs_g

                def pass_a(c):
                    xs_g = xs_tiles[c // GRP]
                    ci = c % GRP
                    vt_sb = p1.tile([128, 2, 384], bf, tag="vt")
                    for wc in range(2):
                        vt_ps = p1vt.tile([128, 384], f32, tag="vtps")
                        for hc in range(2):
                            nc.tensor.matmul(
                                vt_ps[:],
                                lhsT=xs_g[:, hc, ci, wc * 128 : (wc + 1) * 128],
                                rhs=gvt_sb[:, hc, :],
                                start=(hc == 0),
                                stop=(hc == 1),
                            )
                        nc.vector.tensor_copy(vt_sb[:, wc, :], vt_ps[:])
                    vt_tiles[c] = vt_sb

                def pass_b(c):
                    vt_sb = vt_tiles.pop(c)
                    blur_ps = p1bl.tile([128, 768], f32, tag="blps")
                    for s in range(3):
                        for wc in range(2):
                            nc.tensor.matmul(
                                blur_ps[:, s * 256 : (s + 1) * 256],
                                lhsT=vt_sb[:, wc, s * 128 : (s + 1) * 128],
                                rhs=gh_sb[:, s * 2 + wc, :],
                                start=(wc == 0),
                                stop=(wc == 1),
                            )
                    l_all = p1.tile([128, 768], bf, tag="lall")
                    nc.scalar.activation(l_all[:], blur_ps[:], AF.Ln, bias=1.0)
                    l12 = p1.tile([128, 256], bf, tag="l12")
                    nc.vector.tensor_add(l12[:], l_all[:, 0:256], l_all[:, 256:512])
                    nc.sync.dma_start(l12d_v[c], l12[:])
                    nc.gpsimd.dma_start(l3d_v[c], l_all[:, 512:768])

                load_group(0)
                pass_a(0)
                for c in range(C):
                    if (c + 1) % GRP == 0 and c + 1 < C:
                        load_group((c + 1) // GRP)
                    if c + 1 < C:
                        pass_a(c + 1)
                    pass_b(c)

            # ---- phase 2a staging: [g; log1p(x)] conv rhs (lx host-precomputed,
            # so no ACT work here; emitted after phase 1 exactly as in the
            # 234us baseline so phase-1 queue traffic is untouched)
            for q in range(8):
                sl = slice(q * 4096, (q + 1) * 4096)
                nc.sync.dma_start(rhs1_all[0:64, sl], gb_e[:, sl])
                nc.scalar.dma_start(rhs1_all[64:128, sl], lxh_e[:, sl])

            # ---- phase 2b: conv1x1s + relu + psi + sigmoid + multiply
            with (
                tc.tile_pool(name="p2", bufs=2) as p2,
                tc.tile_pool(name="p2ab", bufs=2, space="PSUM") as p2ab,
                tc.tile_pool(name="p2s", bufs=2, space="PSUM") as p2s,
            ):
                rhs2_tiles = {}

                def gather_rhs2(grp):
                    rhs2 = p2.tile([128, GRP, W], bf, tag="rhs2")
                    px = grp * 2048
                    nc.scalar.dma_start(rhs2[0:64], l12d[:, px : px + 2048])
                    nc.sync.dma_start(rhs2[64:128], l3d[:, px : px + 2048])
                    rhs2_tiles[grp] = rhs2

                gather_rhs2(0)
                for grp in range(16):
                    if grp + 1 < 16:
                        gather_rhs2(grp + 1)
                    px = grp * 2048
                    rhs2 = rhs2_tiles.pop(grp)
                    rhs2f = rhs2[:].rearrange("p h w -> p (h w)")
                    ab_ps = p2ab.tile([128, 512], f32, tag="abps")
                    for t in range(4):
                        osl = ab_ps[32 * t : 32 * t + 32, :]
                        nc.tensor.matmul(
                            osl,
                            lhsT=w1_sb[:],
                            rhs=rhs1_all[:, px + 512 * t : px + 512 * (t + 1)],
                            start=True,
                            stop=False,
                            tile_position=(0, 32 * t),
                        )
                        nc.tensor.matmul(
                            osl,
                            lhsT=w2_sb[:],
                            rhs=rhs2f[:, 512 * t : 512 * (t + 1)],
                            start=False,
                            stop=True,
                            tile_position=(0, 32 * t),
                        )
                    relu_sb = p2.tile([128, 512], bf, tag="relu")
                    nc.vector.tensor_scalar(
                        relu_sb[:],
                        ab_ps[:],
                        bias0_sb[:],
                        0.0,
                        mybir.AluOpType.add,
                        mybir.AluOpType.max,
                    )
                    s_ps = p2s.tile([128, 1024], f32, tag="sps")
                    for t in range(4):
                        a, bb = t // 2, t % 2
                        nc.tensor.matmul(
                            s_ps[64 * a : 64 * a + 64, 512 * bb : 512 * bb + 512],
                            lhsT=wpsi_sb[32 * t : 32 * t + 32, :],
                            rhs=relu_sb[32 * t : 32 * t + 32, :],
                            start=True,
                            stop=True,
                            tile_position=(32 * t, 64 * a),
                        )
                    psi_sb = p2.tile([128, 1024], bf, tag="psi")
                    nc.scalar.activation(
                        psi_sb[:], s_ps[:], AF.Sigmoid, bias=bpsi_sb[:]
                    )
                    xb2 = p2.tile([128, 1024], bf, tag="xb2")
                    nc.sync.dma_start(xb2[0:64, :], xh_e[:, px : px + 1024])
                    nc.scalar.dma_start(
                        xb2[64:128, :], xh_e[:, px + 1024 : px + 2048]
                    )
                    out2 = p2.tile([128, 1024], f32, tag="out2")
                    nc.vector.tensor_mul(out2[:], xb2[:], psi_sb[:])
                    nc.gpsimd.dma_start(out_e[:, px : px + 1024], out2[0:64, :])
                    nc.gpsimd.dma_start(
                        out_e[:, px + 1024 : px + 2048], out2[64:128, :]
                    )

    nc.finalize()
    return nc


def kernel(**inputs):
    from concourse.bass_utils import run_bass_kernel_spmd

    g = np.asarray(inputs["g"], dtype=np.float32)
    x = np.asarray(inputs["x"], dtype=np.float32)

    def f(name):
        return np.asarray(inputs[name], dtype=np.float32)

    # Fold eval-mode BN into the 1x1 convs.
    ag = f("wg_gamma") / np.sqrt(f("wg_var") + EPS)
    wg_eff = ag[:, None] * f("wg_w")[:, :, 0, 0]  # [32, 64]
    bg_eff = ag * (f("wg_b") - f("wg_mean")) + f("wg_beta")
    ax = f("wx_gamma") / np.sqrt(f("wx_var") + EPS)
    wx_eff = ax[:, None] * f("wx_w")[:, :, 0, 0]  # [32, 64]
    bx_eff = ax * (f("wx_b") - f("wx_mean")) + f("wx_beta")
    ap_ = f("psi_gamma") / np.sqrt(f("psi_var") + EPS)
    wpsi_eff = ap_[0] * f("psi_w")[0, :, 0, 0]  # [32]
    bpsi = float(ap_[0] * (f("psi_b")[0] - f("psi_mean")[0]) + f("psi_beta")[0])
    bias0 = bg_eff + bx_eff  # [32]

    Gs = [_gauss_mat(s) for s in SCALES]

    # w1: rows 0-63 -> g channels (Wg), 64-127 -> log1p(x) channels (Wx)
    w1 = np.concatenate([wg_eff.T, wx_eff.T], axis=0).astype(BF16)  # [128, 32]
    # w2: rows 0-63 -> l12 channels, 64-127 -> l3 channels, both -Wx/3
    w2 = np.concatenate([-wx_eff.T / 3.0, -wx_eff.T / 3.0], axis=0).astype(BF16)
    wpsi_t = np.broadcast_to(
        np.tile(wpsi_eff, 4)[:, None], (128, 64)
    ).astype(BF16)  # [128, 64]: row 32t+o = wpsi[o], replicated over 64 cols
    bias0_t = np.tile(bias0, 4)[:, None].astype(np.float32)  # [128, 1]
    bpsi_t = np.full((128, 1), bpsi, dtype=np.float32)

    # gh[wp, s*2+wc, w] = G_s[wc*128+wp, w]   (pass-B moving operand, all cores)
    gh = np.empty((128, 6, W), dtype=np.float32)
    for s in range(3):
        for wc in range(2):
            gh[:, s * 2 + wc, :] = Gs[s][wc * 128 : (wc + 1) * 128, :]
    gh = gh.astype(BF16)

    key = "nc"
    if key not in _CACHE:
        _CACHE[key] = _build_nc()
    nc = _CACHE[key]

    in_maps = []
    for core in range(N_CORES):
        b, half = core // 2, core % 2
        h0 = half * HALF
        # gvt[hp, hc, s*128+ho] = G_s[hc*128+hp, h0+ho]  (pass-A moving operand)
        gvt = np.empty((128, 2, 384), dtype=np.float32)
        for hc in range(2):
            for s in range(3):
                gvt[:, hc, s * 128 : (s + 1) * 128] = Gs[s][
                    hc * 128 : (hc + 1) * 128, h0 : h0 + HALF
                ]
        in_maps.append(
            {
                "xbt": np.ascontiguousarray(
                    x[b].reshape(C, 2, HALF, W).transpose(1, 2, 0, 3)
                ).astype(BF16),
                "xh": x[b, :, h0 : h0 + HALF, :].reshape(C, HALF * W).astype(BF16),
                "gb": g[b, :, h0 : h0 + HALF, :].reshape(C, HALF * W).astype(BF16),
                "lxh": np.log1p(x[b, :, h0 : h0 + HALF, :])
                .reshape(C, HALF * W)
                .astype(BF16),
                "gvt": gvt.astype(BF16),
                "gh": gh,
                "w1": w1,
                "w2": w2,
                "wpsi": wpsi_t,
                "bias0": bias0_t,
                "bpsi": bpsi_t,
            }
        )

    global _LAST_IN_MAPS
    _LAST_IN_MAPS = in_maps
    res = run_bass_kernel_spmd(nc, in_maps, core_ids=list(range(N_CORES)))

    out = np.empty((B, C, H, W), dtype=np.float32)
    for core in range(N_CORES):
        b, half = core // 2, core % 2
        h0 = half * HALF
        out[b, :, h0 : h0 + HALF, :] = res.results[core]["out"].reshape(C, HALF, W)
    return out



# revision 6
# speedup vs baseline: 1.1563x; 1.1563x over previous
"""AttentionWithMSR Trainium2 kernel — 8-core SPMD, data-parallel over (batch, H-half).

Self-contained: takes FULL inputs, shards internally, returns FULL output.

Math (reference):
    msr  = log1p(x) - (1/3) * sum_s log1p(blur_s(x)),  s in {15, 80, 250}
    a    = BN(conv1x1(g;  Wg)),  b = BN(conv1x1(msr; Wx))
    psi  = sigmoid(BN(conv1x1(relu(a + b); wpsi)))
    out  = x * psi

Kernel mapping:
  * blur_s(x) per (b, c) image M as two 256x256 matmuls: blur = G_s @ M @ G_s
    (G_s symmetric Toeplitz from the zero-padded normalized 1D Gaussian).
    Each core owns one batch sample b and one 128-row H-half:
      pass A (vertical, transposed): Vt = M^T @ GvT    [w(256, 2 chunks), h_own(128)]
      pass B (horizontal):           Blur = Vt^T @ G_s [h_own(128), w(256)]
  * BN folded into conv weights on host. msr folded into the conv:
      a+b = W1^T @ [g; log1p(x)] + W2^T @ [l12; l3] + bias0,  W2 = -[Wx; Wx]/3.
    l12/l3 round-trip through DRAM to convert per-channel [h, w] tiles into
    [channel, pixel] layout for the channel-contracting conv matmuls.
  * psi conv replicates its scalar output over 64 partitions inside the
    matmul (replicated-weight lhsT) so sigmoid/multiply run full-width.
"""

import sys

sys.path.insert(0, "/opt/trn_rl_repo")

import numpy as np
import ml_dtypes

SCALES = (15, 80, 250)
EPS = 1e-5
B, C, H, W = 4, 64, 256, 256
HALF = 128
FINT = 32
N_CORES = 8
BF16 = ml_dtypes.bfloat16

_CACHE = {}
_LAST_IN_MAPS = None


def _gauss_mat(scale: int) -> np.ndarray:
    """256x256 matrix of the zero-padded 'same' normalized 1D Gaussian blur."""
    k = int(4 * scale + 1)
    p = k // 2
    coords = np.arange(k, dtype=np.float32) - (k - 1) / 2.0
    g1 = np.exp(-(coords**2) / np.float32(2.0 * scale * scale))
    g1 = g1 / g1.sum()
    i = np.arange(W)
    D = i[None, :] - i[:, None]  # j - i
    M = np.where(np.abs(D) <= p, g1[np.clip(D + p, 0, k - 1)], np.float32(0.0))
    return M.astype(np.float32)


def _build_nc():
    import concourse.mybir as mybir
    import concourse.tile as tile
    from concourse import bacc

    bf = mybir.dt.bfloat16
    f32 = mybir.dt.float32
    AF = mybir.ActivationFunctionType

    nc = bacc.Bacc("TRN2", target_bir_lowering=False)

    # x[b] transposed to (h-chunk, h-in-chunk, channel, w) so SBUF loads get
    # 4KB-contiguous per-partition runs.
    xbt_e = nc.dram_tensor("xbt", [2, HALF, C, W], bf, kind="ExternalInput")
    xh_e = nc.dram_tensor("xh", [C, HALF * W], bf, kind="ExternalInput")
    gb_e = nc.dram_tensor("gb", [C, HALF * W], bf, kind="ExternalInput")
    lxh_e = nc.dram_tensor("lxh", [C, HALF * W], bf, kind="ExternalInput")
    gvt_e = nc.dram_tensor("gvt", [128, 2, 384], bf, kind="ExternalInput")
    gh_e = nc.dram_tensor("gh", [128, 6, W], bf, kind="ExternalInput")
    w1_e = nc.dram_tensor("w1", [128, FINT], bf, kind="ExternalInput")
    w2_e = nc.dram_tensor("w2", [128, FINT], bf, kind="ExternalInput")
    wpsi_e = nc.dram_tensor("wpsi", [128, 64], bf, kind="ExternalInput")
    bias0_e = nc.dram_tensor("bias0", [128, 1], f32, kind="ExternalInput")
    bpsi_e = nc.dram_tensor("bpsi", [128, 1], f32, kind="ExternalInput")
    out_e = nc.dram_tensor("out", [C, HALF * W], bf, kind="ExternalOutput")

    NPIX = HALF * W  # 32768 pixels per core

    with tile.TileContext(nc) as tc:
        with (
            tc.tile_pool(name="consts", bufs=1) as consts,
            tc.tile_pool(name="dram", bufs=1, space="DRAM") as dpool,
        ):
            gvt_sb = consts.tile([128, 2, 384], bf)
            nc.sync.dma_start(gvt_sb[:], gvt_e[:])
            gh_sb = consts.tile([128, 6, W], bf)
            nc.sync.dma_start(gh_sb[:], gh_e[:])
            w1_sb = consts.tile([128, FINT], bf)
            nc.sync.dma_start(w1_sb[:], w1_e[:])
            w2_sb = consts.tile([128, FINT], bf)
            nc.sync.dma_start(w2_sb[:], w2_e[:])
            wpsi_sb = consts.tile([128, 64], bf)
            nc.sync.dma_start(wpsi_sb[:], wpsi_e[:])
            bias0_sb = consts.tile([128, 1], f32)
            nc.sync.dma_start(bias0_sb[:], bias0_e[:])
            bpsi_sb = consts.tile([128, 1], f32)
            nc.sync.dma_start(bpsi_sb[:], bpsi_e[:])

            rhs1_all = consts.tile([128, NPIX], bf)  # [g; log1p(x)] conv rhs

            # Stage the phase-2 conv rhs EARLY so the phase-1 -> phase-2
            # boundary never waits on it (4MB, spread over phase 1).
            for q in range(8):
                sl = slice(q * 4096, (q + 1) * 4096)
                nc.sync.dma_start(rhs1_all[0:64, sl], gb_e[:, sl])
                nc.scalar.dma_start(rhs1_all[64:128, sl], lxh_e[:, sl])

            l12d = dpool.tile([C, NPIX], bf)
            l3d = dpool.tile([C, NPIX], bf)
            l12d_v = l12d[:].rearrange("c (h w) -> c h w", h=HALF)
            l3d_v = l3d[:].rearrange("c (h w) -> c h w", h=HALF)

            # ---- phase 1: blur + log1p, software-pipelined over channels
            with (
                tc.tile_pool(name="p1", bufs=3) as p1,
                tc.tile_pool(name="p1x", bufs=8) as p1x,
                tc.tile_pool(name="p1vt", bufs=2, space="PSUM") as p1vt,
                tc.tile_pool(name="p1bl", bufs=3, space="PSUM") as p1bl,
            ):
                GRP = 8  # channels per staged load
                xs_tiles = {}
                vt_tiles = {}

                def load_group(g):
                    xs_g = p1x.tile([128, 2, GRP, W], bf, tag="xs")
                    for hc in range(2):
                        nc.sync.dma_start(
                            xs_g[:, hc], xbt_e[hc, :, g * GRP : (g + 1) * GRP, :]
                        )
                    xs_tiles[g] = xs_g

                def pass_a(c):
                    xs_g = xs_tiles[c // GRP]
                    ci = c % GRP
                    vt_sb = p1.tile([128, 2, 384], bf, tag="vt")
                    for wc in range(2):
                        vt_ps = p1vt.tile([128, 384], f32, tag="vtps")
                        for hc in range(2):
                            nc.tensor.matmul(
                                vt_ps[:],
                                lhsT=xs_g[:, hc, ci, wc * 128 : (wc + 1) * 128],
                                rhs=gvt_sb[:, hc, :],
                                start=(hc == 0),
                                stop=(hc == 1),
                            )
                        nc.vector.tensor_copy(vt_sb[:, wc, :], vt_ps[:])
                    vt_tiles[c] = vt_sb

                def pass_b(c):
                    vt_sb = vt_tiles.pop(c)
                    blur_ps = p1bl.tile([128, 768], f32, tag="blps")
                    for s in range(3):
                        for wc in range(2):
                            nc.tensor.matmul(
                                blur_ps[:, s * 256 : (s + 1) * 256],
                                lhsT=vt_sb[:, wc, s * 128 : (s + 1) * 128],
                                rhs=gh_sb[:, s * 2 + wc, :],
                                start=(wc == 0),
                                stop=(wc == 1),
                            )
                    l_all = p1.tile([128, 768], bf, tag="lall")
                    nc.scalar.activation(l_all[:], blur_ps[:], AF.Ln, bias=1.0)
                    l12 = p1.tile([128, 256], bf, tag="l12")
                    nc.vector.tensor_add(l12[:], l_all[:, 0:256], l_all[:, 256:512])
                    nc.sync.dma_start(l12d_v[c], l12[:])
                    nc.gpsimd.dma_start(l3d_v[c], l_all[:, 512:768])

                # Issue ALL group loads upfront (8MB resident in SBUF) so
                # phase-1 matmuls never wait on a just-in-time prefetch.
                for g in range(C // GRP):
                    load_group(g)
                pass_a(0)
                for c in range(C):
                    if c + 1 < C:
                        pass_a(c + 1)
                    pass_b(c)

            # ---- phase 2b: conv1x1s + relu + psi + sigmoid + multiply
            with (
                tc.tile_pool(name="p2", bufs=2) as p2,
                tc.tile_pool(name="p2r", bufs=3) as p2r,
                tc.tile_pool(name="p2x", bufs=3) as p2x,
                tc.tile_pool(name="p2ab", bufs=2, space="PSUM") as p2ab,
                tc.tile_pool(name="p2s", bufs=2, space="PSUM") as p2s,
            ):
                rhs2_tiles = {}
                xb2_tiles = {}

                def gather_rhs2(grp):
                    rhs2 = p2r.tile([128, GRP, W], bf, tag="rhs2")
                    px = grp * 2048
                    nc.scalar.dma_start(rhs2[0:64], l12d[:, px : px + 2048])
                    nc.sync.dma_start(rhs2[64:128], l3d[:, px : px + 2048])
                    rhs2_tiles[grp] = rhs2

                def load_xb2(grp):
                    px = grp * 2048
                    xb2 = p2x.tile([128, 1024], bf, tag="xb2")
                    nc.sync.dma_start(xb2[0:64, :], xh_e[:, px : px + 1024])
                    nc.scalar.dma_start(
                        xb2[64:128, :], xh_e[:, px + 1024 : px + 2048]
                    )
                    xb2_tiles[grp] = xb2

                gather_rhs2(0)
                gather_rhs2(1)
                load_xb2(0)
                load_xb2(1)
                for grp in range(16):
                    if grp + 2 < 16:
                        gather_rhs2(grp + 2)
                        load_xb2(grp + 2)
                    px = grp * 2048
                    rhs2 = rhs2_tiles.pop(grp)
                    rhs2f = rhs2[:].rearrange("p h w -> p (h w)")
                    ab_ps = p2ab.tile([128, 512], f32, tag="abps")
                    for t in range(4):
                        osl = ab_ps[32 * t : 32 * t + 32, :]
                        nc.tensor.matmul(
                            osl,
                            lhsT=w1_sb[:],
                            rhs=rhs1_all[:, px + 512 * t : px + 512 * (t + 1)],
                            start=True,
                            stop=False,
                            tile_position=(0, 32 * t),
                        )
                        nc.tensor.matmul(
                            osl,
                            lhsT=w2_sb[:],
                            rhs=rhs2f[:, 512 * t : 512 * (t + 1)],
                            start=False,
                            stop=True,
                            tile_position=(0, 32 * t),
                        )
                    relu_sb = p2.tile([128, 512], bf, tag="relu")
                    nc.vector.tensor_scalar(
                        relu_sb[:],
                        ab_ps[:],
                        bias0_sb[:],
                        0.0,
                        mybir.AluOpType.add,
                        mybir.AluOpType.max,
                    )
                    s_ps = p2s.tile([128, 1024], f32, tag="sps")
                    for t in range(4):
                        a, bb = t // 2, t % 2
                        nc.tensor.matmul(
                            s_ps[64 * a : 64 * a + 64, 512 * bb : 512 * bb + 512],
                            lhsT=wpsi_sb[32 * t : 32 * t + 32, :],
                            rhs=relu_sb[32 * t : 32 * t + 32, :],
                            start=True,
                            stop=True,
                            tile_position=(32 * t, 64 * a),
                        )
                    psi_sb = p2.tile([128, 1024], bf, tag="psi")
                    nc.scalar.activation(
                        psi_sb[:], s_ps[:], AF.Sigmoid, bias=bpsi_sb[:]
                    )
                    xb2 = xb2_tiles.pop(grp)
                    out2 = p2.tile([128, 1024], bf, tag="out2")
                    nc.vector.tensor_mul(out2[:], xb2[:], psi_sb[:])
                    nc.gpsimd.dma_start(out_e[:, px : px + 1024], out2[0:64, :])
                    nc.gpsimd.dma_start(
                        out_e[:, px + 1024 : px + 2048], out2[64:128, :]
                    )

    nc.finalize()
    return nc


def kernel(**inputs):
    from concourse.bass_utils import run_bass_kernel_spmd

    g = np.asarray(inputs["g"], dtype=np.float32)
    x = np.asarray(inputs["x"], dtype=np.float32)

    def f(name):
        return np.asarray(inputs[name], dtype=np.float32)

    # Fold eval-mode BN into the 1x1 convs.
    ag = f("wg_gamma") / np.sqrt(f("wg_var") + EPS)
    wg_eff = ag[:, None] * f("wg_w")[:, :, 0, 0]  # [32, 64]
    bg_eff = ag * (f("wg_b") - f("wg_mean")) + f("wg_beta")
    ax = f("wx_gamma") / np.sqrt(f("wx_var") + EPS)
    wx_eff = ax[:, None] * f("wx_w")[:, :, 0, 0]  # [32, 64]
    bx_eff = ax * (f("wx_b") - f("wx_mean")) + f("wx_beta")
    ap_ = f("psi_gamma") / np.sqrt(f("psi_var") + EPS)
    wpsi_eff = ap_[0] * f("psi_w")[0, :, 0, 0]  # [32]
    bpsi = float(ap_[0] * (f("psi_b")[0] - f("psi_mean")[0]) + f("psi_beta")[0])
    bias0 = bg_eff + bx_eff  # [32]

    Gs = [_gauss_mat(s) for s in SCALES]

    # w1: rows 0-63 -> g channels (Wg), 64-127 -> log1p(x) channels (Wx)
    w1 = np.concatenate([wg_eff.T, wx_eff.T], axis=0).astype(BF16)  # [128, 32]
    # w2: rows 0-63 -> l12 channels, 64-127 -> l3 channels, both -Wx/3
    w2 = np.concatenate([-wx_eff.T / 3.0, -wx_eff.T / 3.0], axis=0).astype(BF16)
    wpsi_t = np.broadcast_to(
        np.tile(wpsi_eff, 4)[:, None], (128, 64)
    ).astype(BF16)  # [128, 64]: row 32t+o = wpsi[o], replicated over 64 cols
    bias0_t = np.tile(bias0, 4)[:, None].astype(np.float32)  # [128, 1]
    bpsi_t = np.full((128, 1), bpsi, dtype=np.float32)

    # gh[wp, s*2+wc, w] = G_s[wc*128+wp, w]   (pass-B moving operand, all cores)
    gh = np.empty((128, 6, W), dtype=np.float32)
    for s in range(3):
        for wc in range(2):
            gh[:, s * 2 + wc, :] = Gs[s][wc * 128 : (wc + 1) * 128, :]
    gh = gh.astype(BF16)

    key = "nc"
    if key not in _CACHE:
        _CACHE[key] = _build_nc()
    nc = _CACHE[key]

    in_maps = []
    for core in range(N_CORES):
        b, half = core // 2, core % 2
        h0 = half * HALF
        # gvt[hp, hc, s*128+ho] = G_s[hc*128+hp, h0+ho]  (pass-A moving operand)
        gvt = np.empty((128, 2, 384), dtype=np.float32)
        for hc in range(2):
            for s in range(3):
                gvt[:, hc, s * 128 : (s + 1) * 128] = Gs[s][
                    hc * 128 : (hc + 1) * 128, h0 : h0 + HALF
                ]
        in_maps.append(
            {
                "xbt": np.ascontiguousarray(
                    x[b].reshape(C, 2, HALF, W).transpose(1, 2, 0, 3)
                ).astype(BF16),
                "xh": x[b, :, h0 : h0 + HALF, :].reshape(C, HALF * W).astype(BF16),
                "gb": g[b, :, h0 : h0 + HALF, :].reshape(C, HALF * W).astype(BF16),
                "lxh": np.log1p(x[b, :, h0 : h0 + HALF, :])
                .reshape(C, HALF * W)
                .astype(BF16),
                "gvt": gvt.astype(BF16),
                "gh": gh,
                "w1": w1,
                "w2": w2,
                "wpsi": wpsi_t,
                "bias0": bias0_t,
                "bpsi": bpsi_t,
            }
        )

    global _LAST_IN_MAPS
    _LAST_IN_MAPS = in_maps
    res = run_bass_kernel_spmd(nc, in_maps, core_ids=list(range(N_CORES)))

    out = np.empty((B, C, H, W), dtype=np.float32)
    for core in range(N_CORES):
        b, half = core // 2, core % 2
        h0 = half * HALF
        out[b, :, h0 : h0 + HALF, :] = (
            res.results[core]["out"].astype(np.float32).reshape(C, HALF, W)
        )
    return out

